# revision 43
# baseline (speedup 1.0000x reference)
"""Trainium2 Bass kernel for nn_Attention_7602092114471 (v2, bf16).

Full multi-head attention block:
  qkv = x @ w_qkv.T ; split q,k,v into 12 heads of d=64
  q = rope(q * d**-0.5) ; k = rope(k)   (lucidrains interleaved RoPE)
  attn = softmax(q @ k.T) ; out = (attn @ v) reassembled, @ w_proj.T + b_proj

Shapes: x [2, 2048, 768], w_qkv [2304, 768], w_proj [768, 768], b_proj [768].

Sharding: 24 (batch, head) pairs -> 8 cores x 3 heads. Core c handles batch
c//4, heads {3g, 3g+1, 3g+2} with g = c%4. Each core computes its heads'
q/k/v projections, attention, and a partial output projection over its
3 heads' feature columns. The host sums the 4 partial projections per batch
(the tensor-parallel all-reduce, done on host during unshard) and adds bias.

v2 performance design (all shapes hardcoded):
  * All matmul operands are bf16. On TRN2 the PE streams fp32/fp32r moving
    data at 2 cycles/column but bf16 at 1 (HW: 427ns vs 216ns per 512-wide
    matmul warm) -- bf16 halves all matmul stream time. PSUM stays f32.
  * Scores: [128, 2jb, 512] f32 PSUM tiles; EXP on ScalarE directly out of
    PSUM, 1024 elem/lane per activation, with a constant -8 bias that
    cancels in normalization. No max-subtraction (|S| <~ 10). The 96-act
    exp stream (~103us) is the pacing engine; everything else is arranged
    to keep it gapless.
  * RoPE: q/k are projected feature-major with de-interleaved weight rows
    (host permutation), so the rotate-half partner is p^32 within each
    64-row group. The swap is one PE permutation matmul (p32 stationary)
    per chain-strip; rope = 2 muls + add on DVE (sin-multiply reads the
    swap PSUM directly). Cross-partition-base tensor_tensor with BOTH
    inputs in SBUF is illegal (NCC_IBIR297); one-PSUM-input ops are fine.
  * v is produced token-major directly (stationary = x block, moving = wv
    columns): no PE transposes. Ones columns per head make the PV matmul
    emit the softmax denominator for free (output rows 64..127).
  * Normalization: exact DVE reciprocal + multiply. reciprocal_approx_fast
    (custom DVE uop) returns garbage on HW; gpsimd divide fails the
    Pool-engine ISA check -- neither is usable.
  * Phase A runs qkv chains kt-major in strip-pair waves (double-buffered
    PSUM tags) so chains start as soon as x tiles land; all host weight
    layouts are [partition, ...]-linear so every DMA is contiguous.
  * Phase B is a strip pipeline spliced at emission: scores+exp of strip
    s+1 interleave with PV chunks of strip s and the PROJECTION of strip
    s-1 (delayed a full strip so the 3.4us reciprocal never stalls the
    PE -- a >1.7us PE idle trips the HAM clock gate to 1.2 GHz for ~10us,
    doubling matmul times). PSUM: score 2x2 + pv 3x1 + proj/v 1x1 banks.
  * The host sums the 4 partial projections per batch and adds the bias
    (the tensor-parallel all-reduce, done during unshard).
"""

import numpy as np
import ml_dtypes

import concourse.bass as bass
import concourse.mybir as mybir
import concourse.tile as tile
from concourse import bacc, bass_utils

# Problem constants (hardcoded per contract; kernel.py must be self-contained).
B = 2
N = 2048
C = 768
H = 12
D = 64
ROPE_THETA = 10000.0
NCORES = 8
HPC = 3  # heads per core

F32 = mybir.dt.float32
BF16 = mybir.dt.bfloat16
NPBF16 = ml_dtypes.bfloat16

IS = 512                  # token-strip width (PSUM bank = 512 f32)
NSTRIP = N // IS          # 4
KT = C // 128             # 6 contraction tiles for the projections
NJB = N // 128            # 16 key blocks
EXP_BIAS = -8.0           # constant shift inside exp; cancels in normalization

# Scores in bf16 PSUM (True): 2-bank [128,4,512] score tiles, 2048-elem EXP
# activations. False = f32 PSUM ([128,2,512], 1024-elem EXP). bf16 PSUM
# matmul output is TRN3-only, so this must stay False on TRN2.
SCORE_PSUM_BF16 = False

# Softmax 1/L mode: "exact" = DVE reciprocal. (reciprocal_approx_fast
# mis-executes on HW; gpsimd tensor_tensor divide fails the Pool-engine ISA
# check in walrus. Do not use either.)
RECIP_MODE = "exact"

# Dummy LDWEIGHTS per score group, spread through phase B as PE filler.
# The PE HAM power-state drops to 1.2 GHz after ~1.7us of idle at 2.4 GHz;
# phase B leaves PE ~2.3us/strip idle, which tripped a cold/warm oscillation
# that doubled matmul times. Fillers are free (no PSUM, no output; every
# bass matmul reloads its own weights anyway).
HAM_FILL = 4


def build_nc():
    """Build the per-core Bass module (same NEFF runs SPMD on all 8 cores)."""
    nc = bacc.Bacc(
        "TRN2",
        target_bir_lowering=False,
        debug=False,
        enable_asserts=False,
    )

    xT = nc.dram_tensor("xT", [C, N], BF16, kind="ExternalInput").ap()
    # host-linearized: w_feat[p, kt*576+f] = W[kt*128+p, f]; wp likewise.
    w_feat = nc.dram_tensor(
        "w_feat", [128, KT * 9 * D], BF16, kind="ExternalInput"
    ).ap()
    wp = nc.dram_tensor("wp", [128, 2 * C], BF16, kind="ExternalInput").ap()
    cosT = nc.dram_tensor("cosT", [128, N], BF16, kind="ExternalInput").ap()
    sinT = nc.dram_tensor("sinT", [128, N], BF16, kind="ExternalInput").ap()
    p32 = nc.dram_tensor("p32", [128, 128], BF16, kind="ExternalInput").ap()
    outT = nc.dram_tensor("outT", [C, N], F32, kind="ExternalOutput").ap()

    with tile.TileContext(nc) as tc:
        _kernel_body(tc, nc, xT, w_feat, wp, cosT, sinT, p32, outT)
    nc.compile()
    return nc


def _rope_rows(nc, pool, dst, raw, raw_sw, lo, rows, cos_sb, sin_sb, s):
    """dst[lo:lo+rows, strip s] = raw*cos + swap32(raw)*sinmod.

    All tensor_tensor inputs share base partition `lo` (compiler
    constraint). The sign pattern lives in the sin table; 3 DVE ops,
    all bf16 SBUF (TT 2x mode).
    """
    ss = slice(s * IS, (s + 1) * IS)
    r = slice(lo, lo + rows)
    dsl = dst[r, ss]
    nc.vector.tensor_mul(out=dsl, in0=raw[r, :], in1=cos_sb[r, ss])
    tmp = pool.tile([128, IS], BF16, name="rope_tmp", tag="rope_tmp")
    nc.vector.tensor_mul(out=tmp[r, :], in0=raw_sw[r, :], in1=sin_sb[r, ss])
    nc.vector.tensor_add(out=dsl, in0=dsl, in1=tmp[r, :])


def _kernel_body(tc, nc, xT, w_feat, wp, cosT, sinT, p32, outT):
    import contextlib

    ST_JB = 4 if SCORE_PSUM_BF16 else 2   # j-blocks per score-psum tile
    ST_DT = BF16 if SCORE_PSUM_BF16 else F32

    ctx = contextlib.ExitStack()
    with ctx:
        persist = ctx.enter_context(tc.tile_pool(name="persist", bufs=1))

        # ---- persistent SBUF tensors -------------------------------------
        cos_sb = persist.tile([128, N], BF16, name="cos_sb")
        sin_sb = persist.tile([128, N], BF16, name="sin_sb")

        # roped q/k, feature-major, bf16. q01: heads (h0 | h1); q2d: h2
        # duplicated into both 64-row halves (h2 score matmuls alternate PE
        # row groups so LDWEIGHTS pairs across heads).
        q01 = persist.tile([128, N], BF16, name="q01")
        k01 = persist.tile([128, N], BF16, name="k01")
        q2d = persist.tile([128, N], BF16, name="q2d")
        k2d = persist.tile([128, N], BF16, name="k2d")

        # token-major v with ones columns: group h occupies cols
        # [h*128, h*128+64) = v values, [h*128+64, h*128+128) = 1.0.
        # (memset to 1.0 is emitted right before the v chains; the v copies
        # overwrite the v halves.)
        v_sb = persist.tile([128, NJB, 3 * 128], BF16, name="v_sb")

        # normalized attention outputs, feature-major (PE-ready for proj)
        P0 = persist.tile([128, N], BF16, name="P0")  # heads h0 | h1
        P1 = persist.tile([128, N], BF16, name="P1")  # h2 duplicated

        wp_sb = persist.tile([128, 2, C], BF16, name="wp_sb")

        bias_sb = persist.tile([128, 1], F32, name="bias_sb")
        nc.vector.memset(bias_sb, EXP_BIAS)
        ones64 = persist.tile([64, IS], F32, name="ones64")
        nc.vector.memset(ones64, 1.0)
        # pre-warm the exp table set (~2.6us TABLE_LOAD) during the DMA wait
        warm = persist.tile([128, 1], F32, name="warm")
        nc.scalar.activation(
            out=warm, in_=bias_sb, func=mybir.ActivationFunctionType.Exp
        )

        # ---- phase A: projections + rope (needs x/w in SBUF) -------------
        with tc.tile_pool(name="phA", bufs=1) as phA:
            # DMA order = need order: p32 (first swap mm), per-kt weights
            # and x (chains consume kt-major), cos/sin (first rope), wp
            # (projection, much later). All host-side layouts are already
            # [p, ...] linear so every DMA is contiguous per partition.
            p32_sb = phA.tile([128, 128], BF16, name="p32_sb")
            nc.sync.dma_start(p32_sb, p32)
            w_sb = phA.tile([128, KT, 9 * D], BF16, name="w_sb")
            x_sb = []
            for kt in range(KT):
                nc.sync.dma_start(
                    w_sb[:, kt, :], w_feat[:, kt * 9 * D : (kt + 1) * 9 * D]
                )
                x_t = phA.tile([128, N], BF16, name=f"x_sb{kt}", tag=f"x_sb{kt}")
                nc.sync.dma_start(x_t, xT[kt * 128 : (kt + 1) * 128, :])
                x_sb.append(x_t)
            nc.sync.dma_start(cos_sb, cosT)
            nc.sync.dma_start(sin_sb, sinT)
            nc.sync.dma_start(wp_sb, wp.rearrange("p (o f) -> p o f", o=2))

            with (
                tc.tile_pool(name="rope", bufs=4) as rope_pool,
                tc.tile_pool(name="raw", bufs=6) as raw_pool,
                tc.tile_pool(name="phAps", bufs=2, space="PSUM") as phAps,
                tc.tile_pool(name="swps", bufs=4, space="PSUM") as swps,
            ):
                # w_feat column blocks: 0:128 q0|q1 (de-interleaved,
                # scaled), 128:256 k0|k1, 256:384 q2|k2, 384:576 v0|v1|v2.
                # Chains run kt-major in 4-strip waves so the first matmuls
                # start as soon as x_sb[0] lands (x load is the startup
                # floor), and each (kt, col) stationary is reused across 4
                # strips. Each chain yields raw (bf16 SBUF) and raw_sw =
                # swap32(raw) via a PE permutation matmul (PE has slack
                # here; DVE partition-offset copies were the phase-A
                # bottleneck).
                def chain_wave(col):
                    # strip-PAIR sub-waves with double-buffered psum tags:
                    # the next pair's chains start while this pair's ACT
                    # copies drain (bufs=1 serialized waves on the copies).
                    outs = []
                    for pair in range(NSTRIP // 2):
                        pts = [
                            phAps.tile([128, IS], F32, name="qkv_ps",
                                       tag=f"qkv_ps{i}")
                            for i in range(2)
                        ]
                        for kt in range(KT):
                            for i in range(2):
                                s = 2 * pair + i
                                nc.tensor.matmul(
                                    pts[i],
                                    w_sb[:, kt, col : col + 128],
                                    x_sb[kt][:, s * IS : (s + 1) * IS],
                                    start=(kt == 0),
                                    stop=(kt == KT - 1),
                                )
                        for i in range(2):
                            raw = raw_pool.tile([128, IS], BF16, name="raw",
                                                tag="raw")
                            nc.scalar.copy(out=raw, in_=pts[i])
                            sw_ps = swps.tile([128, IS], F32, name="sw_ps",
                                              tag="sw_ps")
                            nc.tensor.matmul(sw_ps, p32_sb, raw,
                                             start=True, stop=True)
                            # rope's sin-multiply reads sw_ps (PSUM)
                            # directly -- no second ACT copy on the chain.
                            outs.append((raw, sw_ps))
                    return outs

                # K waves first (scores for strip 0 need the full K), q01
                # next. k2 lives at rows 64:128 of its chain, so rope it
                # into k2d[64:128] (tensor_tensor needs equal base
                # partitions) and duplicate down.
                for s, (raw, sw) in enumerate(chain_wave(128)):    # k0|k1
                    _rope_rows(nc, rope_pool, k01, raw, sw, 0, 128,
                               cos_sb, sin_sb, s)
                for s, (raw, sw) in enumerate(chain_wave(256)):    # q2|k2
                    _rope_rows(nc, rope_pool, q2d, raw, sw, 0, 64,
                               cos_sb, sin_sb, s)
                    _rope_rows(nc, rope_pool, k2d, raw, sw, 64, 64,
                               cos_sb, sin_sb, s)
                nc.vector.tensor_copy(out=q2d[64:128, :], in_=q2d[0:64, :])
                nc.vector.tensor_copy(out=k2d[0:64, :], in_=k2d[64:128, :])
                for s, (raw, sw) in enumerate(chain_wave(0)):      # q0|q1
                    _rope_rows(nc, rope_pool, q01, raw, sw, 0, 128,
                               cos_sb, sin_sb, s)

            # ---- phase B: attention + projection, strip-pipelined --------
            # (x_sb/w_sb stay live: the v chains are emitted after strip-0
            # scores so they fill PE score-slot waits instead of delaying
            # the first EXP.)
            with (
                tc.tile_pool(name="epool", bufs=2) as epool,
                tc.tile_pool(name="nrm", bufs=3) as nrm,
                tc.tile_pool(name="prout", bufs=4) as prout,
                tc.tile_pool(name="stps", bufs=2, space="PSUM") as stps,
                tc.tile_pool(name="pvps", bufs=3, space="PSUM") as pvps,
                tc.tile_pool(name="prps", bufs=1, space="PSUM") as prps,
            ):
                def make_e():
                    return [
                        epool.tile([128, NJB, IS], BF16, name=f"e{h}",
                                   tag=f"e{h}")
                        for h in range(HPC)
                    ]

                def se_group(s, jg, e):
                    """Scores + exp for one jg group of a strip."""
                    ss = slice(s * IS, (s + 1) * IS)
                    for h in range(HPC):
                        st = stps.tile([128, ST_JB, IS], ST_DT, name="st",
                                       tag="st")
                        for jj in range(ST_JB):
                            jb = ST_JB * jg + jj
                            jbs = slice(jb * 128, (jb + 1) * 128)
                            half = jb & 1
                            hh = slice(half * 64, half * 64 + 64)
                            srcs = (
                                (k01[0:64, jbs], q01[0:64, ss]),
                                (k01[64:128, jbs], q01[64:128, ss]),
                                (k2d[hh, jbs], q2d[hh, ss]),
                            )[h]
                            nc.tensor.matmul(
                                st[:, jj, :], srcs[0], srcs[1],
                                start=True, stop=True,
                            )
                        nc.scalar.activation(
                            out=e[h][:, ST_JB * jg : ST_JB * (jg + 1), :],
                            in_=st,
                            func=mybir.ActivationFunctionType.Exp,
                            bias=bias_sb[:, :],
                        )
                    for _ in range(HAM_FILL):
                        nc.tensor.ldweights(wp_sb[:, 0, 0:128])

                def v_chains():
                    # v token-major: stationary = x block, moving = wv
                    # columns. PSUM slots shared with the proj pool (1-bank
                    # tiles, WAR-chained; v is long done before proj runs).
                    nc.vector.memset(v_sb, 1.0)
                    for jb in range(NJB):
                        jbs = slice(jb * 128, (jb + 1) * 128)
                        pv = prps.tile([128, IS], F32, name="pp", tag="pp")
                        for kt in range(KT):
                            nc.tensor.matmul(
                                pv[:, 0 : 3 * D],
                                x_sb[kt][:, jbs],
                                w_sb[:, kt, 384 : 384 + 3 * D],
                                start=(kt == 0),
                                stop=(kt == KT - 1),
                            )
                        dst = v_sb[:, jb, :].rearrange(
                            "p (h x) -> p h x", h=3
                        )[:, :, 0:D]
                        nc.vector.tensor_copy(
                            out=dst,
                            in_=pv[:, 0 : 3 * D].rearrange(
                                "p (h x) -> p h x", h=3
                            ),
                        )

                # pv/norm/proj for strip s, split into 8 chunks that are
                # spliced between the score groups of strip s+1 so the PE
                # fills its score-slot waits (st bufs=2 ties PE's score
                # progress to the ACT exp pace) with useful work.
                pv_live = {}

                def chunk(s, e, c):
                    ss = slice(s * IS, (s + 1) * IS)
                    if c < 6:
                        h, part = divmod(c, 2)
                        if part == 0:
                            pv_live[h] = pvps.tile([128, IS], F32,
                                                   name="pv", tag="pv")
                        pv = pv_live[h]
                        for jb in range(8 * part, 8 * part + 8):
                            nc.tensor.matmul(
                                pv,
                                v_sb[:, jb, h * 128 : (h + 1) * 128],
                                e[h][:, jb, :],
                                start=(jb == 0),
                                stop=(jb == NJB - 1),
                            )
                        if part == 1:
                            rt = nrm.tile([128, IS], F32, name="rt", tag="rt")
                            r64 = rt[64:128, :]
                            nc.vector.reciprocal(r64, pv[64:128, :])
                            if h < 2:
                                nc.vector.tensor_mul(
                                    out=P0[h * 64 : h * 64 + 64, ss],
                                    in0=pv[0:64, :], in1=r64,
                                )
                            else:
                                nc.vector.tensor_mul(
                                    out=P1[0:64, ss], in0=pv[0:64, :], in1=r64
                                )
                                nc.vector.tensor_copy(
                                    out=P1[64:128, ss], in_=P1[0:64, ss]
                                )
                    else:
                        for ob in range(3 * (c - 6), 3 * (c - 5)):
                            obs = slice(ob * 128, (ob + 1) * 128)
                            pp = prps.tile([128, IS], F32, name="pp",
                                           tag="pp")
                            nc.tensor.matmul(
                                pp, wp_sb[:, 0, obs], P0[:, ss],
                                start=True, stop=False,
                            )
                            nc.tensor.matmul(
                                pp, wp_sb[:, 1, obs], P1[:, ss],
                                start=False, stop=True,
                            )
                            ot = prout.tile([128, IS], F32, name="ot",
                                            tag="ot")
                            nc.vector.tensor_copy(out=ot, in_=pp)
                            nc.sync.dma_start(outT[obs, ss], ot)

                # Splice layout per SE(s+1): slots jg0..5 carry PV+norm of
                # strip s; slots jg6..7 carry the PROJ of strip s-1 -- the
                # projection is delayed a full strip so the 3.4us
                # reciprocal in its P0/P1 dependency chain can never stall
                # the PE (a 2.4us PE idle trips the HAM clock gate to
                # 1.2 GHz for ~10us, doubling matmul times).
                e_cur = make_e()
                for jg in range(NJB // ST_JB):
                    se_group(0, jg, e_cur)
                v_chains()
                for s in range(NSTRIP):
                    if s + 1 < NSTRIP:
                        e_nxt = make_e()
                        for jg in range(NJB // ST_JB):
                            se_group(s + 1, jg, e_nxt)
                            if jg < 6:
                                chunk(s, e_cur, jg)
                            elif s >= 1:
                                chunk(s - 1, None, jg)
                    else:
                        e_nxt = None
                        for c in range(6):
                            chunk(s, e_cur, c)
                        chunk(s - 1, None, 6)
                        chunk(s - 1, None, 7)
                        chunk(s, None, 6)
                        chunk(s, None, 7)
                    e_cur = e_nxt


# ---------------------------------------------------------------------------
# Host-side sharding / unsharding
# ---------------------------------------------------------------------------

def _rope_tables():
    inv_freq = 1.0 / (ROPE_THETA ** (np.arange(0, D, 2, dtype=np.float64) / D))
    ang = np.arange(N, dtype=np.float64)[None, :] * inv_freq[:, None]  # [32, N]
    cos64 = np.concatenate([np.cos(ang), np.cos(ang)], axis=0)
    sin64 = np.concatenate([-np.sin(ang), np.sin(ang)], axis=0)
    cosT = np.concatenate([cos64, cos64], axis=0).astype(NPBF16)
    sinT = np.concatenate([sin64, sin64], axis=0).astype(NPBF16)
    return cosT, sinT


def make_core_inputs(x, w_qkv, w_proj):
    """Build the 8 per-core input dicts from full inputs."""
    x = np.asarray(x, dtype=np.float32)
    w_qkv = np.asarray(w_qkv, dtype=np.float32)
    w_proj = np.asarray(w_proj, dtype=np.float32)

    cosT, sinT = _rope_tables()
    perm = np.concatenate([np.arange(0, D, 2), np.arange(1, D, 2)])  # de-interleave
    p32 = np.zeros((128, 128), dtype=NPBF16)
    p32[np.arange(128) ^ 32, np.arange(128)] = 1  # rotate-half partner rows
    wq, wk, wv = w_qkv[0:C], w_qkv[C : 2 * C], w_qkv[2 * C : 3 * C]
    scale = np.float32(D ** -0.5)
    wpT = np.ascontiguousarray(w_proj.T)  # [in_features, out_channels]

    in_maps = []
    for c in range(NCORES):
        b, g = divmod(c, 4)
        h0, h1, h2 = 3 * g, 3 * g + 1, 3 * g + 2

        def qrow(h):
            return wq[h * D : (h + 1) * D][perm] * scale

        def krow(h):
            return wk[h * D : (h + 1) * D][perm]

        def vrow(h):
            return wv[h * D : (h + 1) * D]

        blocks = [qrow(h0), qrow(h1)]            # cols 0:128
        blocks += [krow(h0), krow(h1)]           # cols 128:256
        blocks += [qrow(h2), krow(h2)]           # cols 256:384
        blocks += [vrow(h0), vrow(h1), vrow(h2)]  # cols 384:576
        w_feat = np.concatenate(blocks, axis=0).T  # [C, 9*D]
        # linearize to [p, kt*576+f] so the per-kt DMA is contiguous
        w_lin = np.ascontiguousarray(
            w_feat.reshape(KT, 128, 9 * D).transpose(1, 0, 2).reshape(
                128, KT * 9 * D
            )
        )
        wp_rows = np.concatenate(
            [wpT[h0 * D : (h0 + 1) * D], wpT[h1 * D : (h1 + 1) * D],
             0.5 * wpT[h2 * D : (h2 + 1) * D], 0.5 * wpT[h2 * D : (h2 + 1) * D]],
            axis=0,
        )  # [256, C]
        wp_lin = np.ascontiguousarray(
            wp_rows.reshape(2, 128, C).transpose(1, 0, 2).reshape(128, 2 * C)
        )
        in_maps.append(
            {
                "xT": np.ascontiguousarray(x[b].T).astype(NPBF16),
                "w_feat": w_lin.astype(NPBF16),
                "wp": wp_lin.astype(NPBF16),
                "cosT": cosT,
                "sinT": sinT,
                "p32": p32,
            }
        )
    return in_maps


def unshard(core_outs, b_proj):
    """Sum the 4 partial projections per batch, transpose, add bias."""
    b_proj = np.asarray(b_proj, dtype=np.float32)
    out = np.empty((B, N, C), dtype=np.float32)
    for b in range(B):
        acc = core_outs[4 * b].astype(np.float32).copy()
        for g in range(1, 4):
            acc += core_outs[4 * b + g]
        out[b] = acc.T + b_proj
    return out


_NC_CACHE = {}


def get_nc():
    key = (SCORE_PSUM_BF16, RECIP_MODE)
    if key not in _NC_CACHE:
        _NC_CACHE[key] = build_nc()
    return _NC_CACHE[key]


def run(inputs, trace=False, **spmd_kwargs):
    """Run on hardware; returns (full_output, BassKernelResults)."""
    nc = get_nc()
    in_maps = make_core_inputs(inputs["x"], inputs["w_qkv"], inputs["w_proj"])
    res = bass_utils.run_bass_kernel_spmd(
        nc, in_maps, core_ids=list(range(NCORES)), trace=trace, **spmd_kwargs
    )
    core_outs = [r["outT"] for r in res.results]
    return unshard(core_outs, inputs["b_proj"]), res


def kernel(x, w_qkv, w_proj, b_proj):
    out, _ = run({"x": x, "w_qkv": w_qkv, "w_proj": w_proj, "b_proj": b_proj})
    return out


# revision 44
# speedup vs baseline: 1.0574x; 1.0574x over previous
"""Trainium2 Bass kernel for nn_Attention_7602092114471 (v2, bf16).

Full multi-head attention block:
  qkv = x @ w_qkv.T ; split q,k,v into 12 heads of d=64
  q = rope(q * d**-0.5) ; k = rope(k)   (lucidrains interleaved RoPE)
  attn = softmax(q @ k.T) ; out = (attn @ v) reassembled, @ w_proj.T + b_proj

Shapes: x [2, 2048, 768], w_qkv [2304, 768], w_proj [768, 768], b_proj [768].

Sharding: 24 (batch, head) pairs -> 8 cores x 3 heads. Core c handles batch
c//4, heads {3g, 3g+1, 3g+2} with g = c%4. Each core computes its heads'
q/k/v projections, attention, and a partial output projection over its
3 heads' feature columns. The host sums the 4 partial projections per batch
(the tensor-parallel all-reduce, done on host during unshard) and adds bias.

v2 performance design (all shapes hardcoded):
  * All matmul operands are bf16. On TRN2 the PE streams fp32/fp32r moving
    data at 2 cycles/column but bf16 at 1 (HW: 427ns vs 216ns per 512-wide
    matmul warm) -- bf16 halves all matmul stream time. PSUM stays f32.
  * Scores: [128, 2jb, 512] f32 PSUM tiles; EXP on ScalarE directly out of
    PSUM, 1024 elem/lane per activation, with a constant -8 bias that
    cancels in normalization. No max-subtraction (|S| <~ 10). The 96-act
    exp stream (~103us) is the pacing engine; everything else is arranged
    to keep it gapless.
  * RoPE: q/k are projected feature-major with de-interleaved weight rows
    (host permutation), so the rotate-half partner is p^32 within each
    64-row group. The swap is one PE permutation matmul (p32 stationary)
    per chain-strip; rope = 2 muls + add on DVE (sin-multiply reads the
    swap PSUM directly). Cross-partition-base tensor_tensor with BOTH
    inputs in SBUF is illegal (NCC_IBIR297); one-PSUM-input ops are fine.
  * v is produced token-major directly (stationary = x block, moving = wv
    columns): no PE transposes. Ones columns per head make the PV matmul
    emit the softmax denominator for free (output rows 64..127).
  * Normalization: exact DVE reciprocal + multiply. reciprocal_approx_fast
    (custom DVE uop) returns garbage on HW; gpsimd divide fails the
    Pool-engine ISA check -- neither is usable.
  * Phase A runs qkv chains kt-major in strip-pair waves (double-buffered
    PSUM tags) so chains start as soon as x tiles land; all host weight
    layouts are [partition, ...]-linear so every DMA is contiguous.
  * Phase B is a strip pipeline spliced at emission: scores+exp of strip
    s+1 interleave with PV chunks of strip s and the PROJECTION of strip
    s-1 (delayed a full strip so the 3.4us reciprocal never stalls the
    PE -- a >1.7us PE idle trips the HAM clock gate to 1.2 GHz for ~10us,
    doubling matmul times). PSUM: score 2x2 + pv 3x1 + proj/v 1x1 banks.
  * The host sums the 4 partial projections per batch and adds the bias
    (the tensor-parallel all-reduce, done during unshard).
"""

import numpy as np
import ml_dtypes

import concourse.bass as bass
import concourse.mybir as mybir
import concourse.tile as tile
from concourse import bacc, bass_utils

# Problem constants (hardcoded per contract; kernel.py must be self-contained).
B = 2
N = 2048
C = 768
H = 12
D = 64
ROPE_THETA = 10000.0
NCORES = 8
HPC = 3  # heads per core

F32 = mybir.dt.float32
BF16 = mybir.dt.bfloat16
NPBF16 = ml_dtypes.bfloat16

IS = 512                  # token-strip width (PSUM bank = 512 f32)
NSTRIP = N // IS          # 4
KT = C // 128             # 6 contraction tiles for the projections
NJB = N // 128            # 16 key blocks
EXP_BIAS = -8.0           # constant shift inside exp; cancels in normalization

# Scores in bf16 PSUM (True): 2-bank [128,4,512] score tiles, 2048-elem EXP
# activations. False = f32 PSUM ([128,2,512], 1024-elem EXP). bf16 PSUM
# matmul output is TRN3-only, so this must stay False on TRN2.
SCORE_PSUM_BF16 = False

# Softmax 1/L mode: "exact" = DVE reciprocal. (reciprocal_approx_fast
# mis-executes on HW; gpsimd tensor_tensor divide fails the Pool-engine ISA
# check in walrus. Do not use either.)
RECIP_MODE = "exact"

# Dummy LDWEIGHTS per score group, spread through phase B as PE filler.
# The PE HAM power-state drops to 1.2 GHz after ~1.7us of idle at 2.4 GHz;
# phase B leaves PE ~2.3us/strip idle, which tripped a cold/warm oscillation
# that doubled matmul times. Fillers are free (no PSUM, no output; every
# bass matmul reloads its own weights anyway).
HAM_FILL = 4


def build_nc():
    """Build the per-core Bass module (same NEFF runs SPMD on all 8 cores)."""
    nc = bacc.Bacc(
        "TRN2",
        target_bir_lowering=False,
        debug=False,
        enable_asserts=False,
    )

    xT = nc.dram_tensor("xT", [C, N], BF16, kind="ExternalInput").ap()
    # host-linearized: w_feat[p, kt*576+f] = W[kt*128+p, f]; wp likewise.
    w_feat = nc.dram_tensor(
        "w_feat", [128, KT * 9 * D], BF16, kind="ExternalInput"
    ).ap()
    wp = nc.dram_tensor("wp", [128, 2 * C], BF16, kind="ExternalInput").ap()
    cosT = nc.dram_tensor("cosT", [128, N], BF16, kind="ExternalInput").ap()
    sinT = nc.dram_tensor("sinT", [128, N], BF16, kind="ExternalInput").ap()
    p32 = nc.dram_tensor("p32", [128, 128], BF16, kind="ExternalInput").ap()
    outT = nc.dram_tensor("outT", [C, N], F32, kind="ExternalOutput").ap()

    with tile.TileContext(nc) as tc:
        _kernel_body(tc, nc, xT, w_feat, wp, cosT, sinT, p32, outT)
    nc.compile()
    return nc


def _rope_rows(nc, pool, dst, raw, raw_sw, lo, rows, cos_sb, sin_sb, s):
    """dst[lo:lo+rows, strip s] = raw*cos + swap32(raw)*sinmod.

    All tensor_tensor inputs share base partition `lo` (compiler
    constraint). The sign pattern lives in the sin table; 3 DVE ops,
    all bf16 SBUF (TT 2x mode).
    """
    ss = slice(s * IS, (s + 1) * IS)
    r = slice(lo, lo + rows)
    dsl = dst[r, ss]
    nc.vector.tensor_mul(out=dsl, in0=raw[r, :], in1=cos_sb[r, ss])
    tmp = pool.tile([128, IS], BF16, name="rope_tmp", tag="rope_tmp")
    nc.vector.tensor_mul(out=tmp[r, :], in0=raw_sw[r, :], in1=sin_sb[r, ss])
    nc.vector.tensor_add(out=dsl, in0=dsl, in1=tmp[r, :])


def _kernel_body(tc, nc, xT, w_feat, wp, cosT, sinT, p32, outT):
    import contextlib

    ST_JB = 4 if SCORE_PSUM_BF16 else 2   # j-blocks per score-psum tile
    ST_DT = BF16 if SCORE_PSUM_BF16 else F32

    ctx = contextlib.ExitStack()
    with ctx:
        persist = ctx.enter_context(tc.tile_pool(name="persist", bufs=1))

        # ---- persistent SBUF tensors -------------------------------------
        cos_sb = persist.tile([128, N], BF16, name="cos_sb")
        sin_sb = persist.tile([128, N], BF16, name="sin_sb")

        # roped q/k, feature-major, bf16. q01: heads (h0 | h1); q2d: h2
        # duplicated into both 64-row halves (h2 score matmuls alternate PE
        # row groups so LDWEIGHTS pairs across heads).
        q01 = persist.tile([128, N], BF16, name="q01")
        k01 = persist.tile([128, N], BF16, name="k01")
        q2d = persist.tile([128, N], BF16, name="q2d")
        k2d = persist.tile([128, N], BF16, name="k2d")

        # token-major v with ones columns: group h occupies cols
        # [h*128, h*128+64) = v values, [h*128+64, h*128+128) = 1.0.
        # (memset to 1.0 is emitted right before the v chains; the v copies
        # overwrite the v halves.)
        v_sb = persist.tile([128, NJB, 3 * 128], BF16, name="v_sb")

        # normalized attention outputs, feature-major (PE-ready for proj)
        P0 = persist.tile([128, N], BF16, name="P0")  # heads h0 | h1
        P1 = persist.tile([128, N], BF16, name="P1")  # h2 duplicated

        wp_sb = persist.tile([128, 2, C], BF16, name="wp_sb")

        bias_sb = persist.tile([128, 1], F32, name="bias_sb")
        nc.vector.memset(bias_sb, EXP_BIAS)
        ones64 = persist.tile([64, IS], F32, name="ones64")
        nc.vector.memset(ones64, 1.0)
        # pre-warm the exp table set (~2.6us TABLE_LOAD) during the DMA wait
        warm = persist.tile([128, 1], F32, name="warm")
        nc.scalar.activation(
            out=warm, in_=bias_sb, func=mybir.ActivationFunctionType.Exp
        )

        # ---- phase A: projections + rope (needs x/w in SBUF) -------------
        with tc.tile_pool(name="phA", bufs=1) as phA:
            # DMA order = need order: p32 (first swap mm), per-kt weights
            # and x (chains consume kt-major), cos/sin (first rope), wp
            # (projection, much later). All host-side layouts are already
            # [p, ...] linear so every DMA is contiguous per partition.
            p32_sb = phA.tile([128, 128], BF16, name="p32_sb")
            nc.sync.dma_start(p32_sb, p32)
            w_sb = phA.tile([128, KT, 9 * D], BF16, name="w_sb")
            x_sb = []
            for kt in range(KT):
                nc.sync.dma_start(
                    w_sb[:, kt, :], w_feat[:, kt * 9 * D : (kt + 1) * 9 * D]
                )
                x_t = phA.tile([128, N], BF16, name=f"x_sb{kt}", tag=f"x_sb{kt}")
                nc.sync.dma_start(x_t, xT[kt * 128 : (kt + 1) * 128, :])
                x_sb.append(x_t)
            nc.sync.dma_start(cos_sb, cosT)
            nc.sync.dma_start(sin_sb, sinT)
            nc.sync.dma_start(wp_sb, wp.rearrange("p (o f) -> p o f", o=2))

            with (
                tc.tile_pool(name="rope", bufs=4) as rope_pool,
                tc.tile_pool(name="raw", bufs=6) as raw_pool,
                tc.tile_pool(name="phAps", bufs=2, space="PSUM") as phAps,
                tc.tile_pool(name="swps", bufs=4, space="PSUM") as swps,
            ):
                # w_feat column blocks: 0:128 q0|q1 (de-interleaved,
                # scaled), 128:256 k0|k1, 256:384 q2|k2, 384:576 v0|v1|v2.
                # Chains run kt-major in 4-strip waves so the first matmuls
                # start as soon as x_sb[0] lands (x load is the startup
                # floor), and each (kt, col) stationary is reused across 4
                # strips. Each chain yields raw (bf16 SBUF) and raw_sw =
                # swap32(raw) via a PE permutation matmul (PE has slack
                # here; DVE partition-offset copies were the phase-A
                # bottleneck).
                def chain_wave(col):
                    # strip-PAIR sub-waves with double-buffered psum tags:
                    # the next pair's chains start while this pair's ACT
                    # copies drain (bufs=1 serialized waves on the copies).
                    outs = []
                    for pair in range(NSTRIP // 2):
                        pts = [
                            phAps.tile([128, IS], F32, name="qkv_ps",
                                       tag=f"qkv_ps{i}")
                            for i in range(2)
                        ]
                        for kt in range(KT):
                            for i in range(2):
                                s = 2 * pair + i
                                nc.tensor.matmul(
                                    pts[i],
                                    w_sb[:, kt, col : col + 128],
                                    x_sb[kt][:, s * IS : (s + 1) * IS],
                                    start=(kt == 0),
                                    stop=(kt == KT - 1),
                                )
                        for i in range(2):
                            raw = raw_pool.tile([128, IS], BF16, name="raw",
                                                tag="raw")
                            nc.scalar.copy(out=raw, in_=pts[i])
                            sw_ps = swps.tile([128, IS], F32, name="sw_ps",
                                              tag="sw_ps")
                            nc.tensor.matmul(sw_ps, p32_sb, raw,
                                             start=True, stop=True)
                            # rope's sin-multiply reads sw_ps (PSUM)
                            # directly -- no second ACT copy on the chain.
                            outs.append((raw, sw_ps))
                    return outs

                # K waves first (scores for strip 0 need the full K), q01
                # next. k2 lives at rows 64:128 of its chain, so rope it
                # into k2d[64:128] (tensor_tensor needs equal base
                # partitions) and duplicate down.
                for s, (raw, sw) in enumerate(chain_wave(128)):    # k0|k1
                    _rope_rows(nc, rope_pool, k01, raw, sw, 0, 128,
                               cos_sb, sin_sb, s)
                for s, (raw, sw) in enumerate(chain_wave(256)):    # q2|k2
                    _rope_rows(nc, rope_pool, q2d, raw, sw, 0, 64,
                               cos_sb, sin_sb, s)
                    _rope_rows(nc, rope_pool, k2d, raw, sw, 64, 64,
                               cos_sb, sin_sb, s)
                nc.vector.tensor_copy(out=q2d[64:128, :], in_=q2d[0:64, :])
                nc.vector.tensor_copy(out=k2d[0:64, :], in_=k2d[64:128, :])
                for s, (raw, sw) in enumerate(chain_wave(0)):      # q0|q1
                    _rope_rows(nc, rope_pool, q01, raw, sw, 0, 128,
                               cos_sb, sin_sb, s)

            # ---- phase B: attention + projection, strip-pipelined --------
            # (x_sb/w_sb stay live: the v chains are emitted after strip-0
            # scores so they fill PE score-slot waits instead of delaying
            # the first EXP.)
            with (
                tc.tile_pool(name="epool", bufs=2) as epool,
                tc.tile_pool(name="nrm", bufs=3) as nrm,
                tc.tile_pool(name="prout", bufs=4) as prout,
                tc.tile_pool(name="stps", bufs=2, space="PSUM") as stps,
                tc.tile_pool(name="pvps", bufs=2, space="PSUM") as pvps,
                tc.tile_pool(name="prps", bufs=2, space="PSUM") as prps,
            ):
                def make_e():
                    return [
                        epool.tile([128, NJB, IS], BF16, name=f"e{h}",
                                   tag=f"e{h}")
                        for h in range(HPC)
                    ]

                def se_group(s, jg, e):
                    """Scores + exp for one jg group of a strip."""
                    ss = slice(s * IS, (s + 1) * IS)
                    for h in range(HPC):
                        st = stps.tile([128, ST_JB, IS], ST_DT, name="st",
                                       tag="st")
                        for jj in range(ST_JB):
                            jb = ST_JB * jg + jj
                            jbs = slice(jb * 128, (jb + 1) * 128)
                            half = jb & 1
                            hh = slice(half * 64, half * 64 + 64)
                            srcs = (
                                (k01[0:64, jbs], q01[0:64, ss]),
                                (k01[64:128, jbs], q01[64:128, ss]),
                                (k2d[hh, jbs], q2d[hh, ss]),
                            )[h]
                            nc.tensor.matmul(
                                st[:, jj, :], srcs[0], srcs[1],
                                start=True, stop=True,
                            )
                        nc.scalar.activation(
                            out=e[h][:, ST_JB * jg : ST_JB * (jg + 1), :],
                            in_=st,
                            func=mybir.ActivationFunctionType.Exp,
                            bias=bias_sb[:, :],
                        )
                    for _ in range(HAM_FILL):
                        nc.tensor.ldweights(wp_sb[:, 0, 0:128])

                def v_chains():
                    # v token-major: stationary = x block, moving = wv
                    # columns. PSUM slots shared with the proj pool (1-bank
                    # tiles, WAR-chained; v is long done before proj runs).
                    nc.vector.memset(v_sb, 1.0)
                    for jb in range(NJB):
                        jbs = slice(jb * 128, (jb + 1) * 128)
                        pv = prps.tile([128, IS], F32, name="pp", tag="pp")
                        for kt in range(KT):
                            nc.tensor.matmul(
                                pv[:, 0 : 3 * D],
                                x_sb[kt][:, jbs],
                                w_sb[:, kt, 384 : 384 + 3 * D],
                                start=(kt == 0),
                                stop=(kt == KT - 1),
                            )
                        dst = v_sb[:, jb, :].rearrange(
                            "p (h x) -> p h x", h=3
                        )[:, :, 0:D]
                        nc.vector.tensor_copy(
                            out=dst,
                            in_=pv[:, 0 : 3 * D].rearrange(
                                "p (h x) -> p h x", h=3
                            ),
                        )

                # pv/norm/proj for strip s, split into 8 chunks that are
                # spliced between the score groups of strip s+1 so the PE
                # fills its score-slot waits (st bufs=2 ties PE's score
                # progress to the ACT exp pace) with useful work.
                pv_live = {}

                def chunk(s, e, c):
                    ss = slice(s * IS, (s + 1) * IS)
                    if c < 6:
                        h, part = divmod(c, 2)
                        if part == 0:
                            pv_live[h] = pvps.tile([128, IS], F32,
                                                   name="pv", tag="pv")
                        pv = pv_live[h]
                        for jb in range(8 * part, 8 * part + 8):
                            nc.tensor.matmul(
                                pv,
                                v_sb[:, jb, h * 128 : (h + 1) * 128],
                                e[h][:, jb, :],
                                start=(jb == 0),
                                stop=(jb == NJB - 1),
                            )
                        if part == 1:
                            rt = nrm.tile([128, IS], F32, name="rt", tag="rt")
                            r64 = rt[64:128, :]
                            nc.vector.reciprocal(r64, pv[64:128, :])
                            if h < 2:
                                nc.vector.tensor_mul(
                                    out=P0[h * 64 : h * 64 + 64, ss],
                                    in0=pv[0:64, :], in1=r64,
                                )
                            else:
                                nc.vector.tensor_mul(
                                    out=P1[0:64, ss], in0=pv[0:64, :], in1=r64
                                )
                                nc.vector.tensor_copy(
                                    out=P1[64:128, ss], in_=P1[0:64, ss]
                                )
                    else:
                        for ob in range(3 * (c - 6), 3 * (c - 5)):
                            obs = slice(ob * 128, (ob + 1) * 128)
                            pp = prps.tile([128, IS], F32, name="pp",
                                           tag="pp")
                            nc.tensor.matmul(
                                pp, wp_sb[:, 0, obs], P0[:, ss],
                                start=True, stop=False,
                            )
                            nc.tensor.matmul(
                                pp, wp_sb[:, 1, obs], P1[:, ss],
                                start=False, stop=True,
                            )
                            ot = prout.tile([128, IS], F32, name="ot",
                                            tag="ot")
                            nc.vector.tensor_copy(out=ot, in_=pp)
                            nc.sync.dma_start(outT[obs, ss], ot)

                # Splice layout per SE(s+1): slots jg0..5 carry PV+norm of
                # strip s; slots jg6..7 carry the PROJ of strip s-1 -- the
                # projection is delayed a full strip so the 3.4us
                # reciprocal in its P0/P1 dependency chain can never stall
                # the PE (a 2.4us PE idle trips the HAM clock gate to
                # 1.2 GHz for ~10us, doubling matmul times).
                e_cur = make_e()
                for jg in range(NJB // ST_JB):
                    se_group(0, jg, e_cur)
                v_chains()
                for s in range(NSTRIP):
                    if s + 1 < NSTRIP:
                        e_nxt = make_e()
                        for jg in range(NJB // ST_JB):
                            se_group(s + 1, jg, e_nxt)
                            if jg < 6:
                                chunk(s, e_cur, jg)
                            elif s >= 1:
                                chunk(s - 1, None, jg)
                    else:
                        e_nxt = None
                        for c in range(6):
                            chunk(s, e_cur, c)
                        chunk(s - 1, None, 6)
                        chunk(s - 1, None, 7)
                        chunk(s, None, 6)
                        chunk(s, None, 7)
                    e_cur = e_nxt


# ---------------------------------------------------------------------------
# Host-side sharding / unsharding
# ---------------------------------------------------------------------------

def _rope_tables():
    inv_freq = 1.0 / (ROPE_THETA ** (np.arange(0, D, 2, dtype=np.float64) / D))
    ang = np.arange(N, dtype=np.float64)[None, :] * inv_freq[:, None]  # [32, N]
    cos64 = np.concatenate([np.cos(ang), np.cos(ang)], axis=0)
    sin64 = np.concatenate([-np.sin(ang), np.sin(ang)], axis=0)
    cosT = np.concatenate([cos64, cos64], axis=0).astype(NPBF16)
    sinT = np.concatenate([sin64, sin64], axis=0).astype(NPBF16)
    return cosT, sinT


def make_core_inputs(x, w_qkv, w_proj):
    """Build the 8 per-core input dicts from full inputs."""
    x = np.asarray(x, dtype=np.float32)
    w_qkv = np.asarray(w_qkv, dtype=np.float32)
    w_proj = np.asarray(w_proj, dtype=np.float32)

    cosT, sinT = _rope_tables()
    perm = np.concatenate([np.arange(0, D, 2), np.arange(1, D, 2)])  # de-interleave
    p32 = np.zeros((128, 128), dtype=NPBF16)
    p32[np.arange(128) ^ 32, np.arange(128)] = 1  # rotate-half partner rows
    wq, wk, wv = w_qkv[0:C], w_qkv[C : 2 * C], w_qkv[2 * C : 3 * C]
    scale = np.float32(D ** -0.5)
    wpT = np.ascontiguousarray(w_proj.T)  # [in_features, out_channels]

    in_maps = []
    for c in range(NCORES):
        b, g = divmod(c, 4)
        h0, h1, h2 = 3 * g, 3 * g + 1, 3 * g + 2

        def qrow(h):
            return wq[h * D : (h + 1) * D][perm] * scale

        def krow(h):
            return wk[h * D : (h + 1) * D][perm]

        def vrow(h):
            return wv[h * D : (h + 1) * D]

        blocks = [qrow(h0), qrow(h1)]            # cols 0:128
        blocks += [krow(h0), krow(h1)]           # cols 128:256
        blocks += [qrow(h2), krow(h2)]           # cols 256:384
        blocks += [vrow(h0), vrow(h1), vrow(h2)]  # cols 384:576
        w_feat = np.concatenate(blocks, axis=0).T  # [C, 9*D]
        # linearize to [p, kt*576+f] so the per-kt DMA is contiguous
        w_lin = np.ascontiguousarray(
            w_feat.reshape(KT, 128, 9 * D).transpose(1, 0, 2).reshape(
                128, KT * 9 * D
            )
        )
        wp_rows = np.concatenate(
            [wpT[h0 * D : (h0 + 1) * D], wpT[h1 * D : (h1 + 1) * D],
             0.5 * wpT[h2 * D : (h2 + 1) * D], 0.5 * wpT[h2 * D : (h2 + 1) * D]],
            axis=0,
        )  # [256, C]
        wp_lin = np.ascontiguousarray(
            wp_rows.reshape(2, 128, C).transpose(1, 0, 2).reshape(128, 2 * C)
        )
        in_maps.append(
            {
                "xT": np.ascontiguousarray(x[b].T).astype(NPBF16),
                "w_feat": w_lin.astype(NPBF16),
                "wp": wp_lin.astype(NPBF16),
                "cosT": cosT,
                "sinT": sinT,
                "p32": p32,
            }
        )
    return in_maps


def unshard(core_outs, b_proj):
    """Sum the 4 partial projections per batch, transpose, add bias."""
    b_proj = np.asarray(b_proj, dtype=np.float32)
    out = np.empty((B, N, C), dtype=np.float32)
    for b in range(B):
        acc = core_outs[4 * b].astype(np.float32).copy()
        for g in range(1, 4):
            acc += core_outs[4 * b + g]
        out[b] = acc.T + b_proj
    return out


_NC_CACHE = {}


def get_nc():
    key = (SCORE_PSUM_BF16, RECIP_MODE)
    if key not in _NC_CACHE:
        _NC_CACHE[key] = build_nc()
    return _NC_CACHE[key]


def run(inputs, trace=False, **spmd_kwargs):
    """Run on hardware; returns (full_output, BassKernelResults)."""
    nc = get_nc()
    in_maps = make_core_inputs(inputs["x"], inputs["w_qkv"], inputs["w_proj"])
    res = bass_utils.run_bass_kernel_spmd(
        nc, in_maps, core_ids=list(range(NCORES)), trace=trace, **spmd_kwargs
    )
    core_outs = [r["outT"] for r in res.results]
    return unshard(core_outs, inputs["b_proj"]), res


def kernel(x, w_qkv, w_proj, b_proj):
    out, _ = run({"x": x, "w_qkv": w_qkv, "w_proj": w_proj, "b_proj": b_proj})
    return out
